# revision 1
# baseline (speedup 1.0000x reference)
"""Trainium2 Bass kernel for DanceDecoder: 2-layer autoregressive LSTM.

B=8192, T=60, HID=512, OUT=51, LAT=64.  Data-parallel over 8 cores
(1024 batch rows each).  Feature-major layout: features on SBUF
partitions, batch in the free dimension (2 blocks of 512 columns).
Matmuls run in float32r (single-pass fp32, ~TF32 precision) with fp32
PSUM accumulation; the c state and all biases stay fp32.
"""
import sys
sys.path.insert(0, "/opt/trn_rl_repo")

import numpy as np
import concourse.bacc as bacc
import concourse.mybir as mybir
import concourse.tile as tile
from concourse.bass_utils import run_bass_kernel_spmd

HID = 512
OUT = 51
LAT = 64
T = 60
B = 8192
NCORES = 8
BC = B // NCORES          # 1024 batch columns per core
NBLK = 2                  # column blocks per core
NB = BC // NBLK           # 512 columns per block
KH = HID // 128           # 4 hidden chunks

F32 = mybir.dt.float32
F32R = mybir.dt.float32r
AF = mybir.ActivationFunctionType
OP = mybir.AluOpType

_cached = {}


def build_module(unroll_T=T, debug_taps=False, repeat=1):
    nc = bacc.Bacc("TRN2", target_bir_lowering=False, debug=False)

    # ---- DRAM I/O (per core) ----
    zT = nc.dram_tensor("zT", [LAT, BC], F32R, kind="ExternalInput")
    x0 = nc.dram_tensor("x0", [OUT, BC], F32R, kind="ExternalInput")
    wih0T = nc.dram_tensor("wih0T", [OUT, 4 * HID], F32R, kind="ExternalInput")
    whh0T = nc.dram_tensor("whh0T", [HID, 4 * HID], F32R, kind="ExternalInput")
    w1T = nc.dram_tensor("w1T", [2 * HID, 4 * HID], F32R, kind="ExternalInput")
    foutT = nc.dram_tensor("foutT", [HID, OUT], F32R, kind="ExternalInput")
    finitT = nc.dram_tensor("finitT", [LAT, 2 * HID], F32R, kind="ExternalInput")
    b0c = nc.dram_tensor("b0c", [128, 16], F32, kind="ExternalInput")
    b1c = nc.dram_tensor("b1c", [128, 16], F32, kind="ExternalInput")
    binitc = nc.dram_tensor("binitc", [128, 2 * KH], F32, kind="ExternalInput")
    boutc = nc.dram_tensor("boutc", [OUT, 1], F32, kind="ExternalInput")
    frames = nc.dram_tensor("frames", [unroll_T, OUT, BC], F32,
                            kind="ExternalOutput")

    with tile.TileContext(nc) as tc:
        with (
            tc.tile_pool(name="wpool", bufs=1) as wp,
            tc.tile_pool(name="spool", bufs=1) as sp,
            tc.tile_pool(name="tmp", bufs=2) as tp,
            tc.tile_pool(name="psum", bufs=2, space="PSUM") as pp,
        ):
            # ---- persistent SBUF tiles ----
            w_ih0 = wp.tile([OUT, 4 * HID], F32R, tag="w_ih0")
            w_hh0 = wp.tile([128, KH, 4 * HID], F32R, tag="w_hh0")
            w_1 = wp.tile([128, 2 * KH, 4 * HID], F32R, tag="w_1")
            w_out = wp.tile([128, KH, OUT], F32R, tag="w_out")
            w_init = wp.tile([LAT, 2 * HID], F32R, tag="w_init")
            bias0 = wp.tile([128, 16], F32, tag="bias0")
            bias1 = wp.tile([128, 16], F32, tag="bias1")
            biasi = wp.tile([128, 2 * KH], F32, tag="biasi")
            biaso = wp.tile([OUT, 1], F32, tag="biaso")
            z_t = wp.tile([LAT, BC], F32R, tag="z_t")

            h1 = sp.tile([128, KH, BC], F32R, tag="h1")
            c1 = sp.tile([128, KH, BC], F32, tag="c1")
            h2 = sp.tile([128, KH, BC], F32R, tag="h2")
            c2 = sp.tile([128, KH, BC], F32, tag="c2")
            x = sp.tile([OUT, BC], F32R, tag="x")

            # ---- load everything ----
            nc.sync.dma_start(w_ih0[:], wih0T[:])
            for j in range(KH):
                nc.sync.dma_start(w_hh0[:, j, :], whh0T[j * 128:(j + 1) * 128, :])
                nc.sync.dma_start(w_out[:, j, :], foutT[j * 128:(j + 1) * 128, :])
            for j in range(2 * KH):
                nc.sync.dma_start(w_1[:, j, :], w1T[j * 128:(j + 1) * 128, :])
            nc.sync.dma_start(w_init[:], finitT[:])
            nc.sync.dma_start(bias0[:], b0c[:])
            nc.sync.dma_start(bias1[:], b1c[:])
            nc.sync.dma_start(biasi[:], binitc[:])
            nc.sync.dma_start(biaso[:], boutc[:])
            nc.sync.dma_start(z_t[:], zT[:])
            nc.sync.dma_start(x[:], x0[:])

            # ---- init: h0/c0 = fc_init(z), replicated into both layers ----
            def init_pass(rep):
                if rep > 0:
                    nc.sync.dma_start(x[:], x0[:])
                for b in range(NBLK):
                    s = b * NB
                    for m in range(2 * KH):
                        acc = pp.tile([128, NB], F32, tag=f"g{m % 4}")
                        nc.tensor.matmul(acc[:],
                                         w_init[:, m * 128:(m + 1) * 128],
                                         z_t[:, s:s + NB],
                                         start=True, stop=True)
                        if m < KH:
                            dsts = (h1[:, m, s:s + NB], h2[:, m, s:s + NB])
                        else:
                            dsts = (c1[:, m - KH, s:s + NB],
                                    c2[:, m - KH, s:s + NB])
                        nc.vector.tensor_scalar(dsts[0], acc[:],
                                                biasi[:, m:m + 1],
                                                None, OP.add)
                        nc.vector.tensor_copy(dsts[1], dsts[0])

            # ---- one LSTM cell update for (layer, block, chunk) ----
            # Gate results for the new h go to a per-chunk staging tile
            # (hnew); the caller commits them into the state tile only after
            # every matmul of the phase has been traced, so all units read
            # the previous step's h.
            def cell(b, k, c_st, bias, in_mms):
                """in_mms: list of (lhsT_ap, rhs_ap) contraction terms."""
                s = b * NB
                P = {}
                for g in ("i", "g", "f", "o"):
                    gi = {"i": 0, "f": 1, "g": 2, "o": 3}[g]
                    acc = pp.tile([128, NB], F32, tag=f"g{gi}")
                    P[g] = acc
                    col = gi * HID + k * 128
                    n = len(in_mms)
                    for t_, (lhsT, rhs) in enumerate(in_mms):
                        nc.tensor.matmul(acc[:], lhsT[:, col:col + 128], rhs,
                                         start=(t_ == 0), stop=(t_ == n - 1))
                # activations: i/f/o sigmoid in place on PSUM; tanh(g) lands
                # in the SBUF temp (DVE can read at most one PSUM operand).
                ig = tp.tile([128, NB], F32, tag="ig")
                hn = tp.tile([128, NB], F32R, tag=f"hnew{k}")
                nc.scalar.activation(P["i"][:], P["i"][:], AF.Sigmoid,
                                     bias=bias[:, k:k + 1])
                nc.scalar.activation(ig[:], P["g"][:], AF.Tanh,
                                     bias=bias[:, 8 + k:8 + k + 1])
                nc.scalar.activation(P["f"][:], P["f"][:], AF.Sigmoid,
                                     bias=bias[:, 4 + k:4 + k + 1])
                nc.scalar.activation(P["o"][:], P["o"][:], AF.Sigmoid,
                                     bias=bias[:, 12 + k:12 + k + 1])
                cs = c_st[:, k, s:s + NB]
                nc.vector.tensor_tensor(ig[:], P["i"][:], ig[:], OP.mult)
                nc.vector.tensor_tensor(cs, P["f"][:], cs, OP.mult)
                nc.vector.tensor_tensor(cs, cs, ig[:], OP.add)
                nc.scalar.activation(hn[:], cs, AF.Tanh)
                nc.vector.tensor_tensor(hn[:], P["o"][:], hn[:], OP.mult)
                return hn

            # ---- the 60 autoregressive steps ----
            for _rep in range(repeat):
              init_pass(_rep)
              if debug_taps and _rep == 0:
                d_h1i = nc.dram_tensor("d_h1i", [128, KH, BC], F32R,
                                       kind="ExternalOutput")
                d_c1i = nc.dram_tensor("d_c1i", [128, KH, BC], F32,
                                       kind="ExternalOutput")
                nc.sync.dma_start(d_h1i[:], h1[:])
                nc.sync.dma_start(d_c1i[:], c1[:])
              for t in range(unroll_T):
                for b in range(NBLK):
                    s = b * NB
                    hns = []
                    for k in range(KH):
                        mms = [(w_hh0[:, j, :], h1[:, j, s:s + NB])
                               for j in range(KH)]
                        mms.append((w_ih0[:], x[:, s:s + NB]))
                        hns.append(cell(b, k, c1, bias0, mms))
                    for k in range(KH):
                        nc.vector.tensor_copy(h1[:, k, s:s + NB], hns[k][:])
                if debug_taps and t == 0:
                    d_h1s = nc.dram_tensor("d_h1s", [128, KH, BC], F32R,
                                           kind="ExternalOutput")
                    d_c1s = nc.dram_tensor("d_c1s", [128, KH, BC], F32,
                                           kind="ExternalOutput")
                    nc.sync.dma_start(d_h1s[:], h1[:])
                    nc.sync.dma_start(d_c1s[:], c1[:])
                for b in range(NBLK):
                    s = b * NB
                    hns = []
                    for k in range(KH):
                        mms = [(w_1[:, j, :], h1[:, j, s:s + NB])
                               for j in range(KH)]
                        mms += [(w_1[:, KH + j, :], h2[:, j, s:s + NB])
                                for j in range(KH)]
                        hns.append(cell(b, k, c2, bias1, mms))
                    for k in range(KH):
                        nc.vector.tensor_copy(h2[:, k, s:s + NB], hns[k][:])
                for b in range(NBLK):
                    s = b * NB
                    acc = pp.tile([OUT, NB], F32, tag="g0")
                    for j in range(KH):
                        nc.tensor.matmul(acc[:], w_out[:, j, :],
                                         h2[:, j, s:s + NB],
                                         start=(j == 0), stop=(j == KH - 1))
                    nc.vector.tensor_scalar(x[:, s:s + NB], acc[:], biaso[:],
                                            None, OP.add)
                    nc.sync.dma_start(frames[t, :, s:s + NB],
                                      x[:, s:s + NB].bitcast(F32))

    nc.compile()
    return nc


def _prep_inputs(z, start_token, fc_init_w, fc_init_b,
                 w_ih0, w_hh0, b_ih0, b_hh0,
                 w_ih1, w_hh1, b_ih1, b_hh1,
                 fc_out_w, fc_out_b):
    f32 = np.float32
    common = {
        "wih0T": np.ascontiguousarray(w_ih0.T, dtype=f32),
        "whh0T": np.ascontiguousarray(w_hh0.T, dtype=f32),
        "w1T": np.ascontiguousarray(
            np.concatenate([w_ih1.T, w_hh1.T], axis=0), dtype=f32),
        "foutT": np.ascontiguousarray(fc_out_w.T, dtype=f32),
        "finitT": np.ascontiguousarray(fc_init_w.T, dtype=f32),
        "b0c": np.ascontiguousarray(
            (b_ih0 + b_hh0).reshape(4, 4, 128).transpose(2, 0, 1)
            .reshape(128, 16), dtype=f32),
        "b1c": np.ascontiguousarray(
            (b_ih1 + b_hh1).reshape(4, 4, 128).transpose(2, 0, 1)
            .reshape(128, 16), dtype=f32),
        "binitc": np.ascontiguousarray(
            fc_init_b.reshape(2 * KH, 128).T, dtype=f32),
        "boutc": np.ascontiguousarray(fc_out_b[:, None], dtype=f32),
        "x0": np.ascontiguousarray(
            np.broadcast_to(start_token[:, None], (OUT, BC)), dtype=f32),
    }
    in_maps = []
    for c in range(NCORES):
        m = dict(common)
        m["zT"] = np.ascontiguousarray(
            z[c * BC:(c + 1) * BC].T, dtype=f32)
        in_maps.append(m)
    return in_maps


def kernel(**inputs):
    if "nc" not in _cached:
        _cached["nc"] = build_module()
    nc = _cached["nc"]
    in_maps = _prep_inputs(**inputs)
    res = run_bass_kernel_spmd(nc, in_maps, list(range(NCORES)))
    # frames per core: [T, OUT, BC] -> full [B, T, OUT]
    out = np.stack([res.results[c]["frames"] for c in range(NCORES)])
    return np.ascontiguousarray(
        out.transpose(0, 3, 1, 2).reshape(B, T, OUT))



# revision 3
# speedup vs baseline: 6.8147x; 6.8147x over previous
"""Trainium2 Bass kernel for DanceDecoder: 2-layer autoregressive LSTM.

B=8192, T=60, HID=512, OUT=51, LAT=64.  Data-parallel over 8 cores
(1024 batch rows each).  Feature-major layout: features on SBUF
partitions, batch in the free dimension (2 blocks of 512 columns).

v2: recurrent matmuls run in fp8e4m3 with DoubleRow perf mode (2 k-tiles
of 128 per instruction, 0.5 cycles/row), weights pre-scaled by 2^8 and
descaled inside the gate activation's scale parameter.  Gate biases ride
a constant-one fp8 row in the contraction (no per-gate Act bias), which
lets one [128,1536]-wide sigmoid cover i/f/o per cell.  Gate values and
the c state are fp16 so DVE elementwise ops hit the 2x perf mode; h is
staged fp16 then cast to the fp8 state tile on GpSimd.  h1/h2 fp8 state
is double-buffered per step (ping-pong), so no commit copies and no
read-before-write hazards.  fc_out runs in fp16 from the fp16 h2 copy.
"""
import sys
sys.path.insert(0, "/opt/trn_rl_repo")

import numpy as np
import ml_dtypes
import concourse.bacc as bacc
import concourse.mybir as mybir
import concourse.tile as tile
from concourse.bass_utils import run_bass_kernel_spmd

HID = 512
OUT = 51
LAT = 64
T = 60
B = 8192
NCORES = 8
BC = B // NCORES          # 1024 batch columns per core
NBLK = 2                  # column blocks per core
NB = BC // NBLK           # 512 columns per block
KH = HID // 128           # 4 hidden chunks
SW = 256.0                # fp8 weight pre-scale (descaled in activation)

F32 = mybir.dt.float32
F32R = mybir.dt.float32r
F16 = mybir.dt.float16
F8 = mybir.dt.float8e4
AF = mybir.ActivationFunctionType
OP = mybir.AluOpType
DR = mybir.MatmulPerfMode.DoubleRow
E4M3 = ml_dtypes.float8_e4m3

# S-tile (fp8 state) layout along dim1 (20 k-tiles of 128 partitions):
#  0-3  h1 (even steps)    4-7  h1 (odd steps)
#  8    x rows 0-50, ones row 51, zeros 52-127
#  9    zeros (pad partner of tile 8)
# 10-13 h2 (even steps)   14-17 h2 (odd steps)
# 18    ones row 0, zeros elsewhere (L1 bias row)
# 19    zeros (pad partner of tile 18)
NS = 20

_cached = {}


def build_module(unroll_T=T):
    nc = bacc.Bacc("TRN2", target_bir_lowering=False, debug=False)

    # ---- DRAM I/O (per core) ----
    zT = nc.dram_tensor("zT", [LAT, BC], F32R, kind="ExternalInput")
    s0 = nc.dram_tensor("s0", [128, NS, BC], F8, kind="ExternalInput")
    w0d = nc.dram_tensor("w0d", [128, 6, 4 * HID], F8, kind="ExternalInput")
    w1d = nc.dram_tensor("w1d", [128, 10, 4 * HID], F8, kind="ExternalInput")
    wod = nc.dram_tensor("wod", [128, KH, OUT], F16, kind="ExternalInput")
    wid = nc.dram_tensor("wid", [LAT, 2 * HID], F32R, kind="ExternalInput")
    bid = nc.dram_tensor("bid", [128, 8], F32, kind="ExternalInput")
    bod = nc.dram_tensor("bod", [OUT, 1], F32, kind="ExternalInput")
    frames = nc.dram_tensor("frames", [unroll_T, OUT, BC], F32,
                            kind="ExternalOutput")

    with tile.TileContext(nc) as tc:
        with (
            tc.tile_pool(name="wpool", bufs=1) as wp,
            tc.tile_pool(name="tmp", bufs=3) as tp,
            tc.tile_pool(name="psum", bufs=2, space="PSUM") as pp,
        ):
            # ---- persistent SBUF tiles ----
            W0 = wp.tile([128, 6, 4 * HID], F8, tag="W0")
            W1 = wp.tile([128, 10, 4 * HID], F8, tag="W1")
            WO = wp.tile([128, KH, OUT], F16, tag="WO")
            WI = wp.tile([LAT, 2 * HID], F32R, tag="WI")
            S = wp.tile([128, NS, BC], F8, tag="S")
            BI = wp.tile([128, 8], F32, tag="BI")
            BO = wp.tile([OUT, 1], F32, tag="BO")
            ZT = wp.tile([LAT, BC], F32R, tag="ZT")
            C1 = wp.tile([128, KH, BC], F16, tag="C1")
            C2 = wp.tile([128, KH, BC], F16, tag="C2")
            H1F = wp.tile([128, KH, BC], F16, tag="H1F")
            H2F = wp.tile([128, KH, BC], F16, tag="H2F")
            XF = wp.tile([OUT, BC], F32, tag="XF")

            # ---- load everything ----
            nc.sync.dma_start(W0[:], w0d[:])
            nc.sync.dma_start(W1[:], w1d[:])
            nc.sync.dma_start(WO[:], wod[:])
            nc.sync.dma_start(WI[:], wid[:])
            nc.sync.dma_start(S[:], s0[:])
            nc.sync.dma_start(BI[:], bid[:])
            nc.sync.dma_start(BO[:], bod[:])
            nc.sync.dma_start(ZT[:], zT[:])

            # ---- init: h0/c0 = fc_init(z), h0 into both layers' fp8 state,
            # c0 into both fp16 c tiles ----
            for b in range(NBLK):
                s = b * NB
                for m in range(8):
                    acc = pp.tile([128, NB], F32, tag="G1")
                    nc.tensor.matmul(acc[:], WI[:, m * 128:(m + 1) * 128],
                                     ZT[:, s:s + NB], start=True, stop=True)
                    if m < KH:
                        ht = tp.tile([128, NB], F16, tag="ht")
                        nc.vector.tensor_scalar(ht[:], acc[:], BI[:, m:m + 1],
                                                None, OP.add)
                        nc.gpsimd.tensor_copy(S[:, m, s:s + NB], ht[:])
                        nc.gpsimd.tensor_copy(S[:, 10 + m, s:s + NB], ht[:])
                    else:
                        cm = m - KH
                        nc.vector.tensor_scalar(C1[:, cm, s:s + NB], acc[:],
                                                BI[:, m:m + 1], None, OP.add)
                        nc.vector.tensor_copy(C2[:, cm, s:s + NB],
                                              C1[:, cm, s:s + NB])

            # ---- one LSTM cell for (layer, block, k-chunk) at parity par ----
            # DoubleRow matmuls accumulate i/f/o into a 3-bank PSUM tile and
            # g into its own bank; one wide sigmoid + one tanh produce fp16
            # gates in SBUF; the c update runs fp16 on DVE; h goes to the
            # fp16 staging tile and is cast to the fp8 state on GpSimd.
            def cell(layer, b, k, par):
                s = b * NB
                P3 = pp.tile([128, 3 * NB], F32, tag="P3")
                G1 = pp.tile([128, NB], F32, tag="G1")
                if layer == 0:
                    hb = 4 * par          # h1 from previous step
                    pairs = [(W0, 0, hb), (W0, 2, hb + 2), (W0, 4, 8)]
                else:
                    nb_ = 4 * (1 - par)   # h1 written THIS step by layer 0
                    hb = 10 + 4 * par     # h2 from previous step
                    pairs = [(W1, 0, nb_), (W1, 2, nb_ + 2),
                             (W1, 4, hb), (W1, 6, hb + 2),
                             (W1, 8, 18)]
                n = len(pairs)
                for slot in range(4):
                    col = slot * HID + k * 128
                    dst = P3[:, slot * NB:(slot + 1) * NB] if slot < 3 else G1[:]
                    for pi, (W, wt, st) in enumerate(pairs):
                        nc.tensor.matmul(dst, W[:, wt:wt + 2, col:col + 128],
                                         S[:, st:st + 2, s:s + NB],
                                         start=(pi == 0), stop=(pi == n - 1),
                                         perf_mode=DR)
                sifo = tp.tile([128, 3 * NB], F16, tag="sifo")
                gt = tp.tile([128, NB], F16, tag="gt")
                nc.scalar.activation(sifo[:], P3[:], AF.Sigmoid, scale=1.0 / SW)
                nc.scalar.activation(gt[:], G1[:], AF.Tanh, scale=1.0 / SW)
                cs = (C1 if layer == 0 else C2)[:, k, s:s + NB]
                ig = tp.tile([128, NB], F16, tag="ig")
                nc.vector.tensor_tensor(ig[:], sifo[:, 0:NB], gt[:], OP.mult)
                nc.vector.tensor_tensor(cs, sifo[:, NB:2 * NB], cs, OP.mult)
                nc.vector.tensor_tensor(cs, cs, ig[:], OP.add)
                tc_ = tp.tile([128, NB], F16, tag="tc")
                nc.scalar.activation(tc_[:], cs, AF.Tanh)
                hf = (H1F if layer == 0 else H2F)[:, k, s:s + NB]
                nc.vector.tensor_tensor(hf, sifo[:, 2 * NB:3 * NB], tc_[:],
                                        OP.mult)
                base = 4 * (1 - par) if layer == 0 else 10 + 4 * (1 - par)
                nc.gpsimd.tensor_copy(S[:, base + k, s:s + NB], hf)

            # ---- the autoregressive steps ----
            for t in range(unroll_T):
                par = t % 2
                for b in range(NBLK):
                    for k in range(KH):
                        cell(0, b, k, par)
                for b in range(NBLK):
                    for k in range(KH):
                        cell(1, b, k, par)
                for b in range(NBLK):
                    s = b * NB
                    acc = pp.tile([OUT, NB], F32, tag="G1")
                    for j in range(KH):
                        nc.tensor.matmul(acc[:], WO[:, j, :],
                                         H2F[:, j, s:s + NB],
                                         start=(j == 0), stop=(j == KH - 1))
                    nc.vector.tensor_scalar(XF[:, s:s + NB], acc[:], BO[:],
                                            None, OP.add)
                    nc.sync.dma_start(frames[t, :, s:s + NB], XF[:, s:s + NB])
                    nc.gpsimd.tensor_copy(S[0:OUT, 8, s:s + NB],
                                          XF[:, s:s + NB])

    nc.compile()
    return nc


def _q8(x):
    return np.asarray(x, dtype=np.float32).astype(E4M3)


def _prep_inputs(z, start_token, fc_init_w, fc_init_b,
                 w_ih0, w_hh0, b_ih0, b_hh0,
                 w_ih1, w_hh1, b_ih1, b_hh1,
                 fc_out_w, fc_out_b):
    f32 = np.float32
    H4 = 4 * HID
    perm = [0, 1, 3, 2]  # torch gate order i,f,g,o -> slot order i,f,o,g

    def reorder_rows(w):  # [4H, X] -> gate-slot-major rows
        return w.reshape(4, HID, -1)[perm].reshape(H4, -1)

    def hh_tiles(w, ntiles, off=0):
        # w [4H, K] -> fp8 tiles [128, ntiles, 4H] with tile j at off+j
        wr = reorder_rows(w)  # [4H, K]
        K = wr.shape[1]
        out = np.zeros((128, ntiles, H4), dtype=E4M3)
        for j in range(K // 128):
            out[:, off + j, :] = _q8(wr[:, j * 128:(j + 1) * 128].T * SW)
        return out

    # W0: tiles 0-3 = w_hh0; tile 4 rows 0-50 = w_ih0, row 51 = bias; 5 = 0
    w0 = np.zeros((128, 6, H4), dtype=E4M3)
    w0[:, 0:4, :] = hh_tiles(w_hh0, 4)[:, 0:4, :]
    w0[0:OUT, 4, :] = _q8(reorder_rows(w_ih0).T * SW)
    w0[OUT, 4, :] = _q8(reorder_rows((b_ih0 + b_hh0)[:, None])[:, 0] * SW)
    # W1: tiles 0-3 = w_ih1 (vs h1), 4-7 = w_hh1 (vs h2), 8 row0 = bias, 9 = 0
    w1 = np.zeros((128, 10, H4), dtype=E4M3)
    w1[:, 0:4, :] = hh_tiles(w_ih1, 4)[:, 0:4, :]
    w1[:, 4:8, :] = hh_tiles(w_hh1, 4)[:, 0:4, :]
    w1[0, 8, :] = _q8(reorder_rows((b_ih1 + b_hh1)[:, None])[:, 0] * SW)
    # WO fp16 [128, KH, OUT]
    wo = np.zeros((128, KH, OUT), dtype=np.float16)
    for j in range(KH):
        wo[:, j, :] = fc_out_w[:, j * 128:(j + 1) * 128].T.astype(np.float16)
    # s0: x tile 8 (start token rows 0-50, ones row 51), ones tile 18 row 0
    s0 = np.zeros((128, NS, BC), dtype=E4M3)
    s0[0:OUT, 8, :] = _q8(np.broadcast_to(start_token[:, None], (OUT, BC)))
    s0[OUT, 8, :] = E4M3(1.0)
    s0[0, 18, :] = E4M3(1.0)

    common = {
        "w0d": w0,
        "w1d": w1,
        "wod": wo,
        "wid": np.ascontiguousarray(fc_init_w.T, dtype=f32),
        "bid": np.ascontiguousarray(fc_init_b.reshape(8, 128).T, dtype=f32),
        "bod": np.ascontiguousarray(fc_out_b[:, None], dtype=f32),
        "s0": s0,
    }
    in_maps = []
    for c in range(NCORES):
        m = dict(common)
        m["zT"] = np.ascontiguousarray(z[c * BC:(c + 1) * BC].T, dtype=f32)
        in_maps.append(m)
    return in_maps


def kernel(**inputs):
    if "nc" not in _cached:
        _cached["nc"] = build_module()
    nc = _cached["nc"]
    in_maps = _prep_inputs(**inputs)
    res = run_bass_kernel_spmd(nc, in_maps, list(range(NCORES)))
    # frames per core: [T, OUT, BC] -> full [B, T, OUT]
    out = np.stack([res.results[c]["frames"] for c in range(NCORES)])
    return np.ascontiguousarray(
        out.transpose(0, 3, 1, 2).reshape(B, T, OUT))
